# revision 6
# baseline (speedup 1.0000x reference)
"""Trainium2 Bass kernel for nn_NearestEmbedding (vq_codebook) — v3.

reference:
  xn  = BatchNorm1d(x)   (training mode, biased batch stats)
  out = weight[argmin_k ||xn - weight_k||^2]

argmin via maximization of v = 2*xn.w_k - ||w_k||^2 (row-constant dropped).

v is computed at global scale S=2^12 in PSUM:
  v' = xh@wh (fp16 main) + xh8@wl8 + xl8@wh8 (fp8e4m3 DoubleRow crosses,
       0.5 cyc/row => ~2x PE speedup over a 3-term fp16 split)
     + cones@sdig (rank-2 fp16 matmul adding -S*||w_k||^2)
where xh = fp16(2*S*xn), xl = 2*S*xn - xh, wh = fp16(w), wl = w - wh,
xh8 = fp8(xh*2^-8), wl8 = fp8(wl*2^8), xl8 = fp8(xl), wh8 = fp8(wh).
Numpy-verified on the dataset: 0/16384 argmax flips vs the fp32 reference.

Reduction per 2048-wide quarter (all index forms produce sum(k) - k*):
  DVE tensor_reduce(max) from PSUM -> qmax
  q0/q2: DVE scalar_tensor_tensor: sum(1[pq < qmax]*k)
  q1/q3: ACT Sign(qmax - pq) -> {1 below max, 0 at max} (fp16),
         sign*iota on Pool (fp16 mult), ACT Copy+accum_out sums it
Quarter-OUTER loop: w-side prep for quarter q (DMA loads, fp16 cast +
residual, DMA-xbar transposes, fp8 casts, s-digit segment) is emitted
just before the 16 n-tiles of quarter-q matmuls, so w-prep of q+1
overlaps compute of q. Final per-n-tile combine (first-index tie-break,
matching jnp.argmin) selects k*; codebook rows are gathered from DRAM
via indirect DMA (bit-exact output rows).
"""
import sys
sys.path.insert(0, "/opt/trn_rl_repo")
import numpy as np
import concourse.bass as bass
from concourse import bacc
import concourse.mybir as mybir
from concourse.tile import TileContext
from concourse.bass_utils import run_bass_kernel_spmd

F32 = mybir.dt.float32
F16 = mybir.dt.float16
F8 = mybir.dt.float8e4
I32 = mybir.dt.int32
U16 = mybir.dt.uint16
AX = mybir.AxisListType
OP = mybir.AluOpType
ACTF = mybir.ActivationFunctionType
PM = mybir.MatmulPerfMode

NCORES = 8
N, K, D = 16384, 8192, 256
NS = N // NCORES            # 2048 rows per core
NT = NS // 128              # 16 n-tiles
DH = D // 128               # 2 contract halves
KQ = 1024                   # k-quarter (2 psum banks)
NQ = K // KQ                # 8 quarters
NCH = KQ // 512             # 2 matmul chunks per quarter
BN_EPS = 1e-5

S = 4096.0                  # global v scale 2^12
AX8 = 2.0 ** -8             # xh8 = fp8(xh * AX8)
AW8 = 2.0 ** 8              # wl8 = fp8(wl * AW8)
CC = 16384.0                # s-digit matmul carrier

_cache = {}


def _build() -> bass.Bass:
    from concourse.masks import make_identity

    nc = bacc.Bacc("TRN2", target_bir_lowering=False, debug=False, num_devices=NCORES)
    x = nc.dram_tensor("x", [NS, D], F32, kind="ExternalInput")
    w = nc.dram_tensor("w", [K, D], F32, kind="ExternalInput")
    gamma = nc.dram_tensor("gamma", [D], F32, kind="ExternalInput")
    beta = nc.dram_tensor("beta", [D], F32, kind="ExternalInput")
    y = nc.dram_tensor("y", [NS, D], F32, kind="ExternalOutput")

    cc_in = nc.dram_tensor("cc_in", [128, 4], F32)
    cc_out = nc.dram_tensor("cc_out", [NCORES, 128, 4], F32, addr_space="Shared")
    sd_dram = nc.dram_tensor("sd_dram", [2, K], F16)

    wv = w[:, :].rearrange("(t p) d -> t p d", p=128)       # [64, 128, 256]
    xv = x[:, :].rearrange("(t p) d -> t p d", p=128)       # [16, 128, 256]
    yv = y[:, :].rearrange("(t p) d -> p t d", p=128)       # [128, 16, 256]

    with TileContext(nc) as tc:
        with (
            tc.tile_pool(name="const", bufs=1) as constp,
            tc.tile_pool(name="big", bufs=1) as big,
            tc.tile_pool(name="small", bufs=1) as small,
        ):
            ident = constp.tile([128, 128], F32, tag="ident")
            make_identity(nc, ident[:, :])

            # iota16[p, k] = k for k in 0..KQ-1 (exact in fp16 up to 2048)
            iota16 = constp.tile([128, KQ], F16, tag="iota16")
            with tc.tile_pool(name="iotascr", bufs=1) as iotascr:
                iotau = iotascr.tile([128, KQ], U16, tag="iotau")
                nc.gpsimd.iota(iotau, pattern=[[1, KQ]], base=0, channel_multiplier=0)
                nc.vector.tensor_copy(out=iota16, in_=iotau)

            # persistent tiles
            wh16 = big.tile([128, DH, K], F16, tag="wh16")    # fp16(w) transposed
            wh8 = big.tile([128, DH, K], F8, tag="wh8")       # fp8(wh)
            wl8 = big.tile([128, DH, K], F8, tag="wl8")       # fp8(wl*2^8)
            sdig = big.tile([2, K], F16, tag="sdig")          # -S*s/CC digits
            cones = constp.tile([2, 128], F16, tag="cones")
            nc.vector.memset(cones[:, :], CC)
            xh16 = big.tile([128, DH, NS], F16, tag="xh16")   # fp16(2*S*xn) transp
            xh8 = big.tile([128, DH, NS], F8, tag="xh8")      # fp8(xh*2^-8)
            xl8 = big.tile([128, DH, NS], F8, tag="xl8")      # fp8(2*S*xn - xh)
            xT = [big.tile([128, NS], F32, tag=f"xT{h}", name=f"xT{h}") for h in range(DH)]
            junk_d = big.tile([128, KQ], F16, tag="junk_d")
            junk_a = big.tile([128, KQ], F16, tag="junk_a")

            s_nat = small.tile([128, K // 128], F32, tag="s_nat")   # [128, 64]
            stats = small.tile([128, 4], F32, tag="stats")
            tots = small.tile([128, 4], F32, tag="tots")
            gb = small.tile([128, 4], F32, tag="gb")
            bn = small.tile([128, 8], F32, tag="bn")
            qmaxall = small.tile([128, NT * NQ], F32, tag="qmaxall")
            qidxall = small.tile([128, NT * NQ], F32, tag="qidxall")
            # all index forms produce sum(k)-k*; base = q*KQ + sum(k)
            SUMK = float(KQ * (KQ - 1) // 2)
            qcand_base = small.tile([128, NQ], F32, tag="qcb")
            for q in range(NQ):
                nc.vector.memset(qcand_base[:, q : q + 1], float(q * KQ) + SUMK)

            # ---------- x-phase: load, transpose (PE), BN stats ----------
            with (
                tc.tile_pool(name="xload", bufs=4) as xload,
                tc.tile_pool(name="tps", bufs=4, space="PSUM") as tps,
                tc.tile_pool(name="scr2", bufs=2) as scr2,
            ):
                for t in range(NT):
                    xt = xload.tile([128, D], F32, tag="xt")
                    nc.sync.dma_start(out=xt, in_=xv[t])
                    for h in range(DH):
                        pt = tps.tile([128, 128], F32, tag="pt")
                        nc.tensor.transpose(pt, xt[:, h * 128 : (h + 1) * 128], ident[:, :])
                        nc.vector.tensor_copy(
                            out=xT[h][:, t * 128 : (t + 1) * 128], in_=pt
                        )

                for h in range(DH):
                    nc.vector.tensor_reduce(
                        stats[:, h : h + 1], xT[h][:, :], axis=AX.X, op=OP.add
                    )
                    sq2 = scr2.tile([128, NS], F32, tag="sq2")
                    nc.scalar.activation(
                        out=sq2, in_=xT[h][:, :], func=ACTF.Square,
                        accum_out=stats[:, 2 + h : 3 + h],
                    )

                # launch BN stats AllGather (cheaper than AllReduce);
                # the 8 gathered copies are summed locally below
                nc.sync.dma_start(out=cc_in[:, :], in_=stats)
                nc.gpsimd.collective_compute(
                    "AllGather", OP.bypass,
                    replica_groups=[list(range(NCORES))],
                    ins=[cc_in[:, :]], outs=[cc_out[:, :, :]],
                )
                gat = small.tile([128, NCORES, 4], F32, tag="gat")
                nc.sync.dma_start(
                    out=gat, in_=cc_out[:, :, :].rearrange("r p j -> p r j")
                )
                nc.vector.tensor_reduce(
                    tots, gat[:, :, :].rearrange("p r j -> p j r"),
                    axis=AX.X, op=OP.add,
                )

            with (
                tc.tile_pool(name="wload", bufs=8) as wload,
                tc.tile_pool(name="wstage", bufs=10) as wstage,
                tc.tile_pool(name="wlq", bufs=2) as wlq,
                tc.tile_pool(name="sscr", bufs=2) as sscr,
                tc.tile_pool(name="xnp", bufs=1) as xnp,
                tc.tile_pool(name="mpsum", bufs=4, space="PSUM") as mpsum,
                tc.tile_pool(name="spool", bufs=4) as spool,
                tc.tile_pool(name="gpool", bufs=8) as gpool,
                tc.tile_pool(name="qscr", bufs=10) as qscr,
            ):

                def w_tile_load(wq, ti):
                    t = wq * (KQ // 128) + ti
                    wt = wload.tile([128, D], F32, tag="wt")
                    nc.sync.dma_start(out=wt, in_=wv[t])
                    return wt

                def w_tile_proc(wq, ti, wt, wl16Tq):
                    t = wq * (KQ // 128) + ti
                    w16n = wstage.tile([128, D], F16, tag="w16n")
                    nc.gpsimd.tensor_copy(out=w16n, in_=wt)
                    sqw = wstage.tile([128, D], F16, tag="sqw")
                    nc.scalar.activation(
                        out=sqw, in_=wt, func=ACTF.Square,
                        accum_out=s_nat[:, t : t + 1],
                    )
                    wl16n = wstage.tile([128, D], F16, tag="wl16n")
                    nc.gpsimd.tensor_sub(out=wl16n, in0=wt, in1=w16n)
                    ksl = slice(t * 128, (t + 1) * 128)
                    ksq = slice(ti * 128, (ti + 1) * 128)
                    nc.sync.dma_start_transpose(wh16[:, :, ksl], w16n)
                    nc.sync.dma_start_transpose(wl16Tq[:, :, ksq], wl16n)

                def w_bulk(wq, wl16Tq):
                    qsl = slice(wq * KQ, (wq + 1) * KQ)
                    nc.gpsimd.tensor_copy(out=wh8[:, :, qsl], in_=wh16[:, :, qsl])
                    nc.scalar.activation(
                        out=wl8[:, :, qsl], in_=wl16Tq, func=ACTF.Copy, scale=AW8
                    )

                NWT = KQ // 128
                def w_prep(wq):
                    wl16Tq = wlq.tile([128, DH, KQ], F16, tag="wl16Tq")
                    wts = [w_tile_load(wq, ti) for ti in range(NWT)]
                    for ti in range(NWT):
                        w_tile_proc(wq, ti, wts[ti], wl16Tq)
                    w_bulk(wq, wl16Tq)

                def w_sdig(wq):
                    # s digits segment -> DRAM -> sdig columns
                    qsl = slice(wq * KQ, (wq + 1) * KQ)
                    tsl = slice(wq * (KQ // 128), (wq + 1) * (KQ // 128))
                    NS16 = KQ // 128
                    t1 = sscr.tile([128, NS16], F32, tag="t1")
                    nc.vector.tensor_scalar(
                        t1, s_nat[:, tsl], -(S / CC), scalar2=None, op0=OP.mult
                    )
                    d1 = sscr.tile([128, NS16], F32, tag="d1")
                    d116 = sscr.tile([128, NS16], F16, tag="d116")
                    nc.vector.tensor_copy(out=d116, in_=t1)
                    nc.vector.tensor_copy(out=d1, in_=d116)
                    d2 = sscr.tile([128, NS16], F32, tag="d2")
                    nc.vector.tensor_sub(out=d2, in0=t1, in1=d1)
                    for i, dsrc in enumerate((d1, d2)):
                        ps_ = mpsum.tile([128, KQ], F32, tag="pq")
                        nc.tensor.transpose(ps_[0:NS16, 0:128], dsrc[:, :], ident[:, :])
                        dT = sscr.tile([NS16, 128], F16, tag="dT")
                        nc.vector.tensor_copy(out=dT, in_=ps_[0:NS16, 0:128])
                        nc.sync.dma_start(
                            out=sd_dram[i, qsl].rearrange("(t p) -> t p", p=128),
                            in_=dT[:, :],
                        )
                    nc.sync.dma_start(out=sdig[:, qsl], in_=sd_dram[:, qsl])

                # quarter 0 w-prep before the BN wait
                w_prep(0)
                w_sdig(0)

                # ---------- bn math (waits on the collective) ----------
                nc.sync.dma_start(
                    out=gb[:, 0:2], in_=gamma[:].rearrange("(h p) -> p h", p=128)
                )
                nc.sync.dma_start(
                    out=gb[:, 2:4], in_=beta[:].rearrange("(h p) -> p h", p=128)
                )
                mean = bn[:, 0:2]
                var = bn[:, 2:4]
                rstd = bn[:, 4:6]
                scale2 = bn[:, 6:8]
                inv_n = 1.0 / float(N)
                nc.vector.tensor_scalar(mean, tots[:, 0:2], inv_n, scalar2=None, op0=OP.mult)
                nc.vector.tensor_scalar(var, tots[:, 2:4], inv_n, scalar2=None, op0=OP.mult)
                msq = tots[:, 0:2]
                nc.vector.tensor_tensor(out=msq, in0=mean, in1=mean, op=OP.mult)
                nc.vector.tensor_tensor(out=var, in0=var, in1=msq, op=OP.subtract)
                nc.vector.tensor_scalar(var, var, BN_EPS, scalar2=None, op0=OP.add)
                nc.vector.reciprocal(out=var, in_=var)
                nc.scalar.activation(out=rstd, in_=var, func=ACTF.Sqrt)
                # scale2 = 2*S*rstd*gamma ; bias2 = 2*S*beta - mean*scale2
                nc.vector.tensor_tensor(out=scale2, in0=rstd, in1=gb[:, 0:2], op=OP.mult)
                nc.vector.tensor_scalar(scale2, scale2, 2.0 * S, scalar2=None, op0=OP.mult)
                bias2 = gb[:, 2:4]
                nc.vector.tensor_scalar(bias2, bias2, 2.0 * S, scalar2=None, op0=OP.mult)
                mscale = tots[:, 0:2]
                nc.vector.tensor_tensor(out=mscale, in0=mean, in1=scale2, op=OP.mult)
                nc.vector.tensor_tensor(out=bias2, in0=bias2, in1=mscale, op=OP.subtract)

                # x-side casts (DVE + ACT)
                for h in range(DH):
                    xn2 = xnp.tile([128, NS], F32, tag="xn2")
                    nc.vector.tensor_scalar(
                        xn2, xT[h][:, :],
                        scale2[:, h : h + 1], scalar2=bias2[:, h : h + 1],
                        op0=OP.mult, op1=OP.add,
                    )
                    nc.vector.tensor_copy(out=xh16[:, h, :], in_=xn2)
                    nc.vector.tensor_sub(out=xl8[:, h, :], in0=xn2, in1=xh16[:, h, :])
                nc.scalar.activation(out=xh8, in_=xh16, func=ACTF.Copy, scale=AX8)

                # ---------- main loop: quarter-outer ----------
                def combine(nt):
                    csl = slice(nt * NQ, nt * NQ + NQ)
                    qmax = qmaxall[:, csl]
                    qidx = qscr.tile([128, NQ], F32, tag="qidx")
                    m = qscr.tile([128, 1], F32, tag="m")
                    nc.vector.tensor_reduce(m, qmax, axis=AX.X, op=OP.max)
                    # increasing in k*: qidx = qcand - (sum(k) - k*)
                    nc.gpsimd.tensor_tensor(
                        out=qidx, in0=qcand_base, in1=qidxall[:, csl], op=OP.subtract
                    )
                    pen = qscr.tile([128, NQ], F32, tag="pen")
                    nc.vector.tensor_scalar(
                        pen, qmax, m[:, 0:1], scalar2=1e9, op0=OP.is_lt, op1=OP.mult
                    )
                    nc.gpsimd.tensor_tensor(out=qidx, in0=qidx, in1=pen, op=OP.add)
                    idxf = qscr.tile([128, 1], F32, tag="idxf")
                    nc.vector.tensor_reduce(idxf, qidx[:, :], axis=AX.X, op=OP.min)
                    idxi = qscr.tile([128, 1], I32, tag="idxi")
                    nc.vector.tensor_copy(out=idxi, in_=idxf)
                    gath = gpool.tile([128, D], F32, tag="gath")
                    nc.gpsimd.indirect_dma_start(
                        out=gath,
                        out_offset=None,
                        in_=w[:, :],
                        in_offset=bass.IndirectOffsetOnAxis(ap=idxi[:, 0:1], axis=0),
                    )
                    nc.sync.dma_start(out=yv[:, nt, :], in_=gath)

                for q in range(NQ):
                    wl16Tq = None
                    wpend = []
                    if q + 1 < NQ:
                        wl16Tq = wlq.tile([128, DH, KQ], F16, tag="wl16Tq")
                        wpend = [w_tile_load(q + 1, 0), w_tile_load(q + 1, 1)]

                    NWT_ = KQ // 128
                    for nt in range(NT):
                        if wl16Tq is not None:
                            if nt + 2 < NWT_:
                                wpend.append(w_tile_load(q + 1, nt + 2))
                            if nt < NWT_:
                                w_tile_proc(q + 1, nt, wpend[nt], wl16Tq)
                        nsl = slice(nt * 128, (nt + 1) * 128)
                        col = nt * NQ + q
                        pq = mpsum.tile([128, KQ], F32, tag="pq")
                        i = 0
                        # fp16 main term, term-major (stationary reuse)
                        for h in range(DH):
                            for c in range(NCH):
                                kofs = q * KQ + c * 512
                                nc.tensor.matmul(
                                    pq[:, c * 512 : (c + 1) * 512],
                                    xh16[:, h, nsl],
                                    wh16[:, h, kofs : kofs + 512],
                                    start=(i == 0), stop=False,
                                )
                            i += 1
                        # fp8 DoubleRow crosses (contract 256 per instr)
                        for xa, wa in ((xh8, wl8), (xl8, wh8)):
                            for c in range(NCH):
                                kofs = q * KQ + c * 512
                                nc.tensor.matmul(
                                    pq[:, c * 512 : (c + 1) * 512],
                                    xa[:, :, nsl],
                                    wa[:, :, kofs : kofs + 512],
                                    start=False, stop=False,
                                    perf_mode=PM.DoubleRow,
                                )
                        # -S*||w||^2 rank-2 fp16 matmul
                        for c in range(NCH):
                            kofs = q * KQ + c * 512
                            nc.tensor.matmul(
                                pq[:, c * 512 : (c + 1) * 512],
                                cones[:, :],
                                sdig[:, kofs : kofs + 512],
                                start=False, stop=(c == NCH - 1),
                            )

                        # qmax = max(pq) (DVE, from PSUM)
                        nc.vector.tensor_reduce(
                            qmaxall[:, col : col + 1], pq, axis=AX.X, op=OP.max
                        )
                        # index: qidx = sum(k) - k*
                        if (3 * (nt + q)) % 8 < 3:
                            nc.vector.scalar_tensor_tensor(
                                out=junk_d, in0=pq,
                                scalar=qmaxall[:, col : col + 1],
                                in1=iota16,
                                op0=OP.is_lt, op1=OP.mult,
                                accum_out=qidxall[:, col : col + 1],
                            )
                        else:
                            sgn = spool.tile([128, KQ], F16, tag="sgn")
                            nc.scalar.activation(
                                out=sgn, in_=pq, func=ACTF.Sign,
                                bias=qmaxall[:, col : col + 1], scale=-1.0,
                            )
                            t16 = spool.tile([128, KQ], F16, tag="t16")
                            nc.gpsimd.tensor_tensor(
                                out=t16, in0=sgn, in1=iota16, op=OP.mult
                            )
                            nc.scalar.activation(
                                out=junk_a, in_=t16, func=ACTF.Copy,
                                accum_out=qidxall[:, col : col + 1],
                            )
                        if q == NQ - 1:
                            combine(nt)

                    if q + 1 < NQ:
                        w_bulk(q + 1, wl16Tq)
                        w_sdig(q + 1)


    return nc


def _get_nc():
    if "nc" not in _cache:
        nc_ = _build()
        if not nc_.is_finalized():
            nc_.finalize()
        _cache["nc"] = nc_
    return _cache["nc"]


def kernel(x, weight, gamma, beta):
    x = np.ascontiguousarray(x, dtype=np.float32)
    weight = np.ascontiguousarray(weight, dtype=np.float32)
    gamma = np.ascontiguousarray(gamma, dtype=np.float32)
    beta = np.ascontiguousarray(beta, dtype=np.float32)

    nc = _get_nc()
    in_maps = [
        {
            "x": x[c * NS : (c + 1) * NS],
            "w": weight,
            "gamma": gamma,
            "beta": beta,
        }
        for c in range(NCORES)
    ]
    res = run_bass_kernel_spmd(nc, in_maps, list(range(NCORES)))
    return np.concatenate([res.results[c]["y"] for c in range(NCORES)], axis=0)


if __name__ == "__main__":
    _build()
    print("kernel build OK")


# revision 7
# speedup vs baseline: 1.0040x; 1.0040x over previous
"""Trainium2 Bass kernel for nn_NearestEmbedding (vq_codebook) — v3.

reference:
  xn  = BatchNorm1d(x)   (training mode, biased batch stats)
  out = weight[argmin_k ||xn - weight_k||^2]

argmin via maximization of v = 2*xn.w_k - ||w_k||^2 (row-constant dropped).

v is computed at global scale S=2^12 in PSUM:
  v' = xh@wh (fp16 main) + xh8@wl8 + xl8@wh8 (fp8e4m3 DoubleRow crosses,
       0.5 cyc/row => ~2x PE speedup over a 3-term fp16 split)
     + cones@sdig (rank-2 fp16 matmul adding -S*||w_k||^2)
where xh = fp16(2*S*xn), xl = 2*S*xn - xh, wh = fp16(w), wl = w - wh,
xh8 = fp8(xh*2^-8), wl8 = fp8(wl*2^8), xl8 = fp8(xl), wh8 = fp8(wh).
Numpy-verified on the dataset: 0/16384 argmax flips vs the fp32 reference.

Reduction per 2048-wide quarter (all index forms produce sum(k) - k*):
  DVE tensor_reduce(max) from PSUM -> qmax
  q0/q2: DVE scalar_tensor_tensor: sum(1[pq < qmax]*k)
  q1/q3: ACT Sign(qmax - pq) -> {1 below max, 0 at max} (fp16),
         sign*iota on Pool (fp16 mult), ACT Copy+accum_out sums it
Quarter-OUTER loop: w-side prep for quarter q (DMA loads, fp16 cast +
residual, DMA-xbar transposes, fp8 casts, s-digit segment) is emitted
just before the 16 n-tiles of quarter-q matmuls, so w-prep of q+1
overlaps compute of q. Final per-n-tile combine (first-index tie-break,
matching jnp.argmin) selects k*; codebook rows are gathered from DRAM
via indirect DMA (bit-exact output rows).
"""
import sys
sys.path.insert(0, "/opt/trn_rl_repo")
import numpy as np
import concourse.bass as bass
from concourse import bacc
import concourse.mybir as mybir
from concourse.tile import TileContext
from concourse.bass_utils import run_bass_kernel_spmd

F32 = mybir.dt.float32
F16 = mybir.dt.float16
F8 = mybir.dt.float8e4
I32 = mybir.dt.int32
U16 = mybir.dt.uint16
AX = mybir.AxisListType
OP = mybir.AluOpType
ACTF = mybir.ActivationFunctionType
PM = mybir.MatmulPerfMode

NCORES = 8
N, K, D = 16384, 8192, 256
NS = N // NCORES            # 2048 rows per core
NT = NS // 128              # 16 n-tiles
DH = D // 128               # 2 contract halves
KQ = 1024                   # k-quarter (2 psum banks)
NQ = K // KQ                # 8 quarters
NCH = KQ // 512             # 2 matmul chunks per quarter
BN_EPS = 1e-5

S = 4096.0                  # global v scale 2^12
AX8 = 2.0 ** -8             # xh8 = fp8(xh * AX8)
AW8 = 2.0 ** 8              # wl8 = fp8(wl * AW8)
CC = 16384.0                # s-digit matmul carrier

_cache = {}


def _build() -> bass.Bass:
    from concourse.masks import make_identity

    nc = bacc.Bacc("TRN2", target_bir_lowering=False, debug=False, num_devices=NCORES)
    x = nc.dram_tensor("x", [NS, D], F32, kind="ExternalInput")
    w = nc.dram_tensor("w", [K, D], F32, kind="ExternalInput")
    gamma = nc.dram_tensor("gamma", [D], F32, kind="ExternalInput")
    beta = nc.dram_tensor("beta", [D], F32, kind="ExternalInput")
    y = nc.dram_tensor("y", [NS, D], F32, kind="ExternalOutput")

    cc_in = nc.dram_tensor("cc_in", [128, 4], F32)
    cc_out = nc.dram_tensor("cc_out", [NCORES, 128, 4], F32, addr_space="Shared")
    sd_dram = nc.dram_tensor("sd_dram", [2, K], F16)

    wv = w[:, :].rearrange("(t p) d -> t p d", p=128)       # [64, 128, 256]
    xv = x[:, :].rearrange("(t p) d -> t p d", p=128)       # [16, 128, 256]
    yv = y[:, :].rearrange("(t p) d -> p t d", p=128)       # [128, 16, 256]

    with TileContext(nc) as tc:
        with (
            tc.tile_pool(name="const", bufs=1) as constp,
            tc.tile_pool(name="big", bufs=1) as big,
            tc.tile_pool(name="small", bufs=1) as small,
        ):
            ident = constp.tile([128, 128], F32, tag="ident")
            make_identity(nc, ident[:, :])

            # iota16[p, k] = k for k in 0..KQ-1 (exact in fp16 up to 2048)
            iota16 = constp.tile([128, KQ], F16, tag="iota16")
            with tc.tile_pool(name="iotascr", bufs=1) as iotascr:
                iotau = iotascr.tile([128, KQ], U16, tag="iotau")
                nc.gpsimd.iota(iotau, pattern=[[1, KQ]], base=0, channel_multiplier=0)
                nc.vector.tensor_copy(out=iota16, in_=iotau)

            # persistent tiles
            wh16 = big.tile([128, DH, K], F16, tag="wh16")    # fp16(w) transposed
            wh8 = big.tile([128, DH, K], F8, tag="wh8")       # fp8(wh)
            wl8 = big.tile([128, DH, K], F8, tag="wl8")       # fp8(wl*2^8)
            sdig = big.tile([2, K], F16, tag="sdig")          # -S*s/CC digits
            cones = constp.tile([2, 128], F16, tag="cones")
            nc.vector.memset(cones[:, :], CC)
            c256 = constp.tile([128, DH, KQ], F16, tag="c256")
            nc.vector.memset(c256[:, :, :], AW8)
            xh16 = big.tile([128, DH, NS], F16, tag="xh16")   # fp16(2*S*xn) transp
            xh8 = big.tile([128, DH, NS], F8, tag="xh8")      # fp8(xh*2^-8)
            xl8 = big.tile([128, DH, NS], F8, tag="xl8")      # fp8(2*S*xn - xh)
            xT = [big.tile([128, NS], F32, tag=f"xT{h}", name=f"xT{h}") for h in range(DH)]
            junk_d = big.tile([128, KQ], F16, tag="junk_d")
            junk_a = big.tile([128, KQ], F16, tag="junk_a")

            s_nat = small.tile([128, K // 128], F32, tag="s_nat")   # [128, 64]
            stats = small.tile([128, 4], F32, tag="stats")
            tots = small.tile([128, 4], F32, tag="tots")
            gb = small.tile([128, 4], F32, tag="gb")
            bn = small.tile([128, 8], F32, tag="bn")
            qmaxall = small.tile([128, NT * NQ], F32, tag="qmaxall")
            qidxall = small.tile([128, NT * NQ], F32, tag="qidxall")
            # all index forms produce sum(k)-k*; base = q*KQ + sum(k)
            SUMK = float(KQ * (KQ - 1) // 2)
            qcand_base = small.tile([128, NQ], F32, tag="qcb")
            for q in range(NQ):
                nc.vector.memset(qcand_base[:, q : q + 1], float(q * KQ) + SUMK)

            # ---------- x-phase: load, transpose (PE), BN stats ----------
            with (
                tc.tile_pool(name="xload", bufs=4) as xload,
                tc.tile_pool(name="tps", bufs=4, space="PSUM") as tps,
                tc.tile_pool(name="scr2", bufs=2) as scr2,
            ):
                for t in range(NT):
                    xt = xload.tile([128, D], F32, tag="xt")
                    nc.sync.dma_start(out=xt, in_=xv[t])
                    for h in range(DH):
                        pt = tps.tile([128, 128], F32, tag="pt")
                        nc.tensor.transpose(pt, xt[:, h * 128 : (h + 1) * 128], ident[:, :])
                        nc.vector.tensor_copy(
                            out=xT[h][:, t * 128 : (t + 1) * 128], in_=pt
                        )

                for h in range(DH):
                    nc.vector.tensor_reduce(
                        stats[:, h : h + 1], xT[h][:, :], axis=AX.X, op=OP.add
                    )
                    sq2 = scr2.tile([128, NS], F32, tag="sq2")
                    nc.scalar.activation(
                        out=sq2, in_=xT[h][:, :], func=ACTF.Square,
                        accum_out=stats[:, 2 + h : 3 + h],
                    )

                # launch BN stats AllGather (cheaper than AllReduce);
                # the 8 gathered copies are summed locally below
                nc.sync.dma_start(out=cc_in[:, :], in_=stats)
                nc.gpsimd.collective_compute(
                    "AllGather", OP.bypass,
                    replica_groups=[list(range(NCORES))],
                    ins=[cc_in[:, :]], outs=[cc_out[:, :, :]],
                )
                gat = small.tile([128, NCORES, 4], F32, tag="gat")
                nc.sync.dma_start(
                    out=gat, in_=cc_out[:, :, :].rearrange("r p j -> p r j")
                )
                nc.vector.tensor_reduce(
                    tots, gat[:, :, :].rearrange("p r j -> p j r"),
                    axis=AX.X, op=OP.add,
                )

            with (
                tc.tile_pool(name="wload", bufs=8) as wload,
                tc.tile_pool(name="wstage", bufs=10) as wstage,
                tc.tile_pool(name="wlq", bufs=2) as wlq,
                tc.tile_pool(name="sscr", bufs=2) as sscr,
                tc.tile_pool(name="xnp", bufs=1) as xnp,
                tc.tile_pool(name="mpsum", bufs=4, space="PSUM") as mpsum,
                tc.tile_pool(name="spool", bufs=4) as spool,
                tc.tile_pool(name="gpool", bufs=8) as gpool,
                tc.tile_pool(name="qscr", bufs=10) as qscr,
            ):

                def w_tile_load(wq, ti):
                    t = wq * (KQ // 128) + ti
                    wt = wload.tile([128, D], F32, tag="wt")
                    nc.sync.dma_start(out=wt, in_=wv[t])
                    return wt

                def w_tile_proc(wq, ti, wt, wl16Tq):
                    t = wq * (KQ // 128) + ti
                    w16n = wstage.tile([128, D], F16, tag="w16n")
                    nc.gpsimd.tensor_copy(out=w16n, in_=wt)
                    sqw = wstage.tile([128, D], F16, tag="sqw")
                    nc.scalar.activation(
                        out=sqw, in_=wt, func=ACTF.Square,
                        accum_out=s_nat[:, t : t + 1],
                    )
                    wl16n = wstage.tile([128, D], F16, tag="wl16n")
                    nc.gpsimd.tensor_sub(out=wl16n, in0=wt, in1=w16n)
                    ksl = slice(t * 128, (t + 1) * 128)
                    ksq = slice(ti * 128, (ti + 1) * 128)
                    nc.sync.dma_start_transpose(wh16[:, :, ksl], w16n)
                    nc.sync.dma_start_transpose(wl16Tq[:, :, ksq], wl16n)

                def w_bulk(wq, wl16Tq):
                    qsl = slice(wq * KQ, (wq + 1) * KQ)
                    nc.gpsimd.tensor_copy(out=wh8[:, :, qsl], in_=wh16[:, :, qsl])
                    nc.gpsimd.tensor_tensor(
                        out=wl8[:, :, qsl], in0=wl16Tq, in1=c256, op=OP.mult
                    )

                NWT = KQ // 128
                def w_prep(wq):
                    wl16Tq = wlq.tile([128, DH, KQ], F16, tag="wl16Tq")
                    wts = [w_tile_load(wq, ti) for ti in range(NWT)]
                    for ti in range(NWT):
                        w_tile_proc(wq, ti, wts[ti], wl16Tq)
                    w_bulk(wq, wl16Tq)

                def w_sdig(wq):
                    # s digits segment -> DRAM -> sdig columns
                    qsl = slice(wq * KQ, (wq + 1) * KQ)
                    tsl = slice(wq * (KQ // 128), (wq + 1) * (KQ // 128))
                    NS16 = KQ // 128
                    t1 = sscr.tile([128, NS16], F32, tag="t1")
                    nc.vector.tensor_scalar(
                        t1, s_nat[:, tsl], -(S / CC), scalar2=None, op0=OP.mult
                    )
                    d1 = sscr.tile([128, NS16], F32, tag="d1")
                    d116 = sscr.tile([128, NS16], F16, tag="d116")
                    nc.vector.tensor_copy(out=d116, in_=t1)
                    nc.vector.tensor_copy(out=d1, in_=d116)
                    d2 = sscr.tile([128, NS16], F32, tag="d2")
                    nc.vector.tensor_sub(out=d2, in0=t1, in1=d1)
                    for i, dsrc in enumerate((d1, d2)):
                        ps_ = mpsum.tile([128, KQ], F32, tag="pq")
                        nc.tensor.transpose(ps_[0:NS16, 0:128], dsrc[:, :], ident[:, :])
                        dT = sscr.tile([NS16, 128], F16, tag="dT")
                        nc.vector.tensor_copy(out=dT, in_=ps_[0:NS16, 0:128])
                        nc.sync.dma_start(
                            out=sd_dram[i, qsl].rearrange("(t p) -> t p", p=128),
                            in_=dT[:, :],
                        )
                    nc.sync.dma_start(out=sdig[:, qsl], in_=sd_dram[:, qsl])

                # quarter 0 w-prep before the BN wait
                w_prep(0)
                w_sdig(0)

                # ---------- bn math (waits on the collective) ----------
                nc.sync.dma_start(
                    out=gb[:, 0:2], in_=gamma[:].rearrange("(h p) -> p h", p=128)
                )
                nc.sync.dma_start(
                    out=gb[:, 2:4], in_=beta[:].rearrange("(h p) -> p h", p=128)
                )
                mean = bn[:, 0:2]
                var = bn[:, 2:4]
                rstd = bn[:, 4:6]
                scale2 = bn[:, 6:8]
                inv_n = 1.0 / float(N)
                nc.vector.tensor_scalar(mean, tots[:, 0:2], inv_n, scalar2=None, op0=OP.mult)
                nc.vector.tensor_scalar(var, tots[:, 2:4], inv_n, scalar2=None, op0=OP.mult)
                msq = tots[:, 0:2]
                nc.vector.tensor_tensor(out=msq, in0=mean, in1=mean, op=OP.mult)
                nc.vector.tensor_tensor(out=var, in0=var, in1=msq, op=OP.subtract)
                nc.vector.tensor_scalar(var, var, BN_EPS, scalar2=None, op0=OP.add)
                nc.vector.reciprocal(out=var, in_=var)
                nc.scalar.activation(out=rstd, in_=var, func=ACTF.Sqrt)
                # scale2 = 2*S*rstd*gamma ; bias2 = 2*S*beta - mean*scale2
                nc.vector.tensor_tensor(out=scale2, in0=rstd, in1=gb[:, 0:2], op=OP.mult)
                nc.vector.tensor_scalar(scale2, scale2, 2.0 * S, scalar2=None, op0=OP.mult)
                bias2 = gb[:, 2:4]
                nc.vector.tensor_scalar(bias2, bias2, 2.0 * S, scalar2=None, op0=OP.mult)
                mscale = tots[:, 0:2]
                nc.vector.tensor_tensor(out=mscale, in0=mean, in1=scale2, op=OP.mult)
                nc.vector.tensor_tensor(out=bias2, in0=bias2, in1=mscale, op=OP.subtract)

                # x-side casts (DVE + ACT)
                for h in range(DH):
                    xn2 = xnp.tile([128, NS], F32, tag="xn2")
                    nc.vector.tensor_scalar(
                        xn2, xT[h][:, :],
                        scale2[:, h : h + 1], scalar2=bias2[:, h : h + 1],
                        op0=OP.mult, op1=OP.add,
                    )
                    nc.vector.tensor_copy(out=xh16[:, h, :], in_=xn2)
                    nc.vector.tensor_sub(out=xl8[:, h, :], in0=xn2, in1=xh16[:, h, :])
                nc.scalar.activation(out=xh8, in_=xh16, func=ACTF.Copy, scale=AX8)

                # ---------- main loop: quarter-outer ----------
                def combine(nt):
                    csl = slice(nt * NQ, nt * NQ + NQ)
                    qmax = qmaxall[:, csl]
                    qidx = qscr.tile([128, NQ], F32, tag="qidx")
                    m = qscr.tile([128, 1], F32, tag="m")
                    nc.vector.tensor_reduce(m, qmax, axis=AX.X, op=OP.max)
                    # increasing in k*: qidx = qcand - (sum(k) - k*)
                    nc.gpsimd.tensor_tensor(
                        out=qidx, in0=qcand_base, in1=qidxall[:, csl], op=OP.subtract
                    )
                    pen = qscr.tile([128, NQ], F32, tag="pen")
                    nc.vector.tensor_scalar(
                        pen, qmax, m[:, 0:1], scalar2=1e9, op0=OP.is_lt, op1=OP.mult
                    )
                    nc.gpsimd.tensor_tensor(out=qidx, in0=qidx, in1=pen, op=OP.add)
                    idxf = qscr.tile([128, 1], F32, tag="idxf")
                    nc.vector.tensor_reduce(idxf, qidx[:, :], axis=AX.X, op=OP.min)
                    idxi = qscr.tile([128, 1], I32, tag="idxi")
                    nc.vector.tensor_copy(out=idxi, in_=idxf)
                    gath = gpool.tile([128, D], F32, tag="gath")
                    nc.gpsimd.indirect_dma_start(
                        out=gath,
                        out_offset=None,
                        in_=w[:, :],
                        in_offset=bass.IndirectOffsetOnAxis(ap=idxi[:, 0:1], axis=0),
                    )
                    nc.sync.dma_start(out=yv[:, nt, :], in_=gath)

                for q in range(NQ):
                    wl16Tq = None
                    wpend = []
                    if q + 1 < NQ:
                        wl16Tq = wlq.tile([128, DH, KQ], F16, tag="wl16Tq")
                        wpend = [w_tile_load(q + 1, 0), w_tile_load(q + 1, 1)]

                    NWT_ = KQ // 128
                    for nt in range(NT):
                        if wl16Tq is not None:
                            if nt + 2 < NWT_:
                                wpend.append(w_tile_load(q + 1, nt + 2))
                            if nt < NWT_:
                                w_tile_proc(q + 1, nt, wpend[nt], wl16Tq)
                        nsl = slice(nt * 128, (nt + 1) * 128)
                        col = nt * NQ + q
                        pq = mpsum.tile([128, KQ], F32, tag="pq")
                        i = 0
                        # fp16 main term, term-major (stationary reuse)
                        for h in range(DH):
                            for c in range(NCH):
                                kofs = q * KQ + c * 512
                                nc.tensor.matmul(
                                    pq[:, c * 512 : (c + 1) * 512],
                                    xh16[:, h, nsl],
                                    wh16[:, h, kofs : kofs + 512],
                                    start=(i == 0), stop=False,
                                )
                            i += 1
                        # fp8 DoubleRow crosses (contract 256 per instr)
                        for xa, wa in ((xh8, wl8), (xl8, wh8)):
                            for c in range(NCH):
                                kofs = q * KQ + c * 512
                                nc.tensor.matmul(
                                    pq[:, c * 512 : (c + 1) * 512],
                                    xa[:, :, nsl],
                                    wa[:, :, kofs : kofs + 512],
                                    start=False, stop=False,
                                    perf_mode=PM.DoubleRow,
                                )
                        # -S*||w||^2 rank-2 fp16 matmul
                        for c in range(NCH):
                            kofs = q * KQ + c * 512
                            nc.tensor.matmul(
                                pq[:, c * 512 : (c + 1) * 512],
                                cones[:, :],
                                sdig[:, kofs : kofs + 512],
                                start=False, stop=(c == NCH - 1),
                            )

                        # qmax = max(pq) (DVE, from PSUM)
                        nc.vector.tensor_reduce(
                            qmaxall[:, col : col + 1], pq, axis=AX.X, op=OP.max
                        )
                        # index: qidx = sum(k) - k*
                        if (3 * (nt + q)) % 8 < 3:
                            nc.vector.scalar_tensor_tensor(
                                out=junk_d, in0=pq,
                                scalar=qmaxall[:, col : col + 1],
                                in1=iota16,
                                op0=OP.is_lt, op1=OP.mult,
                                accum_out=qidxall[:, col : col + 1],
                            )
                        else:
                            sgn = spool.tile([128, KQ], F16, tag="sgn")
                            nc.scalar.activation(
                                out=sgn, in_=pq, func=ACTF.Sign,
                                bias=qmaxall[:, col : col + 1], scale=-1.0,
                            )
                            t16 = spool.tile([128, KQ], F16, tag="t16")
                            nc.gpsimd.tensor_tensor(
                                out=t16, in0=sgn, in1=iota16, op=OP.mult
                            )
                            nc.scalar.activation(
                                out=junk_a, in_=t16, func=ACTF.Copy,
                                accum_out=qidxall[:, col : col + 1],
                            )
                        if q == NQ - 1:
                            combine(nt)

                    if q + 1 < NQ:
                        w_bulk(q + 1, wl16Tq)
                        w_sdig(q + 1)


    return nc


def _get_nc():
    if "nc" not in _cache:
        nc_ = _build()
        if not nc_.is_finalized():
            nc_.finalize()
        _cache["nc"] = nc_
    return _cache["nc"]


def kernel(x, weight, gamma, beta):
    x = np.ascontiguousarray(x, dtype=np.float32)
    weight = np.ascontiguousarray(weight, dtype=np.float32)
    gamma = np.ascontiguousarray(gamma, dtype=np.float32)
    beta = np.ascontiguousarray(beta, dtype=np.float32)

    nc = _get_nc()
    in_maps = [
        {
            "x": x[c * NS : (c + 1) * NS],
            "w": weight,
            "gamma": gamma,
            "beta": beta,
        }
        for c in range(NCORES)
    ]
    res = run_bass_kernel_spmd(nc, in_maps, list(range(NCORES)))
    return np.concatenate([res.results[c]["y"] for c in range(NCORES)], axis=0)


if __name__ == "__main__":
    _build()
    print("kernel build OK")


# revision 8
# speedup vs baseline: 1.0060x; 1.0019x over previous
"""Trainium2 Bass kernel for nn_NearestEmbedding (vq_codebook) — v3.

reference:
  xn  = BatchNorm1d(x)   (training mode, biased batch stats)
  out = weight[argmin_k ||xn - weight_k||^2]

argmin via maximization of v = 2*xn.w_k - ||w_k||^2 (row-constant dropped).

v is computed at global scale S=2^12 in PSUM:
  v' = xh@wh (fp16 main) + xh8@wl8 + xl8@wh8 (fp8e4m3 DoubleRow crosses,
       0.5 cyc/row => ~2x PE speedup over a 3-term fp16 split)
     + cones@sdig (rank-2 fp16 matmul adding -S*||w_k||^2)
where xh = fp16(2*S*xn), xl = 2*S*xn - xh, wh = fp16(w), wl = w - wh,
xh8 = fp8(xh*2^-8), wl8 = fp8(wl*2^8), xl8 = fp8(xl), wh8 = fp8(wh).
Numpy-verified on the dataset: 0/16384 argmax flips vs the fp32 reference.

Reduction per 2048-wide quarter (all index forms produce sum(k) - k*):
  DVE tensor_reduce(max) from PSUM -> qmax
  q0/q2: DVE scalar_tensor_tensor: sum(1[pq < qmax]*k)
  q1/q3: ACT Sign(qmax - pq) -> {1 below max, 0 at max} (fp16),
         sign*iota on Pool (fp16 mult), ACT Copy+accum_out sums it
Quarter-OUTER loop: w-side prep for quarter q (DMA loads, fp16 cast +
residual, DMA-xbar transposes, fp8 casts, s-digit segment) is emitted
just before the 16 n-tiles of quarter-q matmuls, so w-prep of q+1
overlaps compute of q. Final per-n-tile combine (first-index tie-break,
matching jnp.argmin) selects k*; codebook rows are gathered from DRAM
via indirect DMA (bit-exact output rows).
"""
import sys
sys.path.insert(0, "/opt/trn_rl_repo")
import numpy as np
import concourse.bass as bass
from concourse import bacc
import concourse.mybir as mybir
from concourse.tile import TileContext
from concourse.bass_utils import run_bass_kernel_spmd

F32 = mybir.dt.float32
F16 = mybir.dt.float16
F8 = mybir.dt.float8e4
I32 = mybir.dt.int32
U16 = mybir.dt.uint16
AX = mybir.AxisListType
OP = mybir.AluOpType
ACTF = mybir.ActivationFunctionType
PM = mybir.MatmulPerfMode

NCORES = 8
N, K, D = 16384, 8192, 256
NS = N // NCORES            # 2048 rows per core
NT = NS // 128              # 16 n-tiles
DH = D // 128               # 2 contract halves
KQ = 1024                   # k-quarter (2 psum banks)
NQ = K // KQ                # 8 quarters
NCH = KQ // 512             # 2 matmul chunks per quarter
BN_EPS = 1e-5

S = 4096.0                  # global v scale 2^12
AX8 = 2.0 ** -8             # xh8 = fp8(xh * AX8)
AW8 = 2.0 ** 8              # wl8 = fp8(wl * AW8)
CC = 16384.0                # s-digit matmul carrier

_cache = {}


def _build() -> bass.Bass:
    from concourse.masks import make_identity

    nc = bacc.Bacc("TRN2", target_bir_lowering=False, debug=False, num_devices=NCORES)
    x = nc.dram_tensor("x", [NS, D], F32, kind="ExternalInput")
    w = nc.dram_tensor("w", [K, D], F32, kind="ExternalInput")
    gamma = nc.dram_tensor("gamma", [D], F32, kind="ExternalInput")
    beta = nc.dram_tensor("beta", [D], F32, kind="ExternalInput")
    y = nc.dram_tensor("y", [NS, D], F32, kind="ExternalOutput")

    cc_in = nc.dram_tensor("cc_in", [128, 4], F32)
    cc_out = nc.dram_tensor("cc_out", [NCORES, 128, 4], F32, addr_space="Shared")
    sd_dram = nc.dram_tensor("sd_dram", [2, K], F16)

    wv = w[:, :].rearrange("(t p) d -> t p d", p=128)       # [64, 128, 256]
    xv = x[:, :].rearrange("(t p) d -> t p d", p=128)       # [16, 128, 256]
    yv = y[:, :].rearrange("(t p) d -> p t d", p=128)       # [128, 16, 256]

    with TileContext(nc) as tc:
        with (
            tc.tile_pool(name="const", bufs=1) as constp,
            tc.tile_pool(name="big", bufs=1) as big,
            tc.tile_pool(name="small", bufs=1) as small,
        ):
            ident = constp.tile([128, 128], F32, tag="ident")
            make_identity(nc, ident[:, :])

            # iota16[p, k] = k for k in 0..KQ-1 (exact in fp16 up to 2048)
            iota16 = constp.tile([128, KQ], F16, tag="iota16")
            with tc.tile_pool(name="iotascr", bufs=1) as iotascr:
                iotau = iotascr.tile([128, KQ], U16, tag="iotau")
                nc.gpsimd.iota(iotau, pattern=[[1, KQ]], base=0, channel_multiplier=0)
                nc.vector.tensor_copy(out=iota16, in_=iotau)

            # persistent tiles
            wh16 = big.tile([128, DH, K], F16, tag="wh16")    # fp16(w) transposed
            wh8 = big.tile([128, DH, K], F8, tag="wh8")       # fp8(wh)
            wl8 = big.tile([128, DH, K], F8, tag="wl8")       # fp8(wl*2^8)
            sdig = big.tile([2, K], F16, tag="sdig")          # -S*s/CC digits
            cones = constp.tile([2, 128], F16, tag="cones")
            nc.vector.memset(cones[:, :], CC)
            c256 = constp.tile([128, DH, KQ], F16, tag="c256")
            nc.vector.memset(c256[:, :, :], AW8)
            xh16 = big.tile([128, DH, NS], F16, tag="xh16")   # fp16(2*S*xn) transp
            xh8 = big.tile([128, DH, NS], F8, tag="xh8")      # fp8(xh*2^-8)
            xl8 = big.tile([128, DH, NS], F8, tag="xl8")      # fp8(2*S*xn - xh)
            xT = [big.tile([128, NS], F32, tag=f"xT{h}", name=f"xT{h}") for h in range(DH)]
            junk_d = big.tile([128, KQ], F16, tag="junk_d")
            junk_a = big.tile([128, KQ], F16, tag="junk_a")

            s_nat = small.tile([128, K // 128], F32, tag="s_nat")   # [128, 64]
            stats = small.tile([128, 4], F32, tag="stats")
            tots = small.tile([128, 4], F32, tag="tots")
            gb = small.tile([128, 4], F32, tag="gb")
            bn = small.tile([128, 8], F32, tag="bn")
            qmaxall = small.tile([128, NT * NQ], F32, tag="qmaxall")
            qidxall = small.tile([128, NT * NQ], F32, tag="qidxall")
            # all index forms produce sum(k)-k*; base = q*KQ + sum(k)
            SUMK = float(KQ * (KQ - 1) // 2)
            qcand_base = small.tile([128, NQ], F32, tag="qcb")
            for q in range(NQ):
                nc.vector.memset(qcand_base[:, q : q + 1], float(q * KQ) + SUMK)

            # ---------- x-phase: load, transpose (PE), BN stats ----------
            with (
                tc.tile_pool(name="xload", bufs=4) as xload,
                tc.tile_pool(name="tps", bufs=4, space="PSUM") as tps,
                tc.tile_pool(name="scr2", bufs=2) as scr2,
            ):
                for t in range(NT):
                    xt = xload.tile([128, D], F32, tag="xt")
                    nc.sync.dma_start(out=xt, in_=xv[t])
                    for h in range(DH):
                        pt = tps.tile([128, 128], F32, tag="pt")
                        nc.tensor.transpose(pt, xt[:, h * 128 : (h + 1) * 128], ident[:, :])
                        nc.vector.tensor_copy(
                            out=xT[h][:, t * 128 : (t + 1) * 128], in_=pt
                        )

                for h in range(DH):
                    nc.vector.tensor_reduce(
                        stats[:, h : h + 1], xT[h][:, :], axis=AX.X, op=OP.add
                    )
                    sq2 = scr2.tile([128, NS], F32, tag="sq2")
                    nc.scalar.activation(
                        out=sq2, in_=xT[h][:, :], func=ACTF.Square,
                        accum_out=stats[:, 2 + h : 3 + h],
                    )

                # launch BN stats AllGather (cheaper than AllReduce);
                # the 8 gathered copies are summed locally below
                nc.sync.dma_start(out=cc_in[:, :], in_=stats)
                nc.gpsimd.collective_compute(
                    "AllGather", OP.bypass,
                    replica_groups=[list(range(NCORES))],
                    ins=[cc_in[:, :]], outs=[cc_out[:, :, :]],
                )
                gat = small.tile([128, NCORES, 4], F32, tag="gat")
                nc.sync.dma_start(
                    out=gat, in_=cc_out[:, :, :].rearrange("r p j -> p r j")
                )
                nc.vector.tensor_reduce(
                    tots, gat[:, :, :].rearrange("p r j -> p j r"),
                    axis=AX.X, op=OP.add,
                )

            with (
                tc.tile_pool(name="wload", bufs=8) as wload,
                tc.tile_pool(name="wstage", bufs=10) as wstage,
                tc.tile_pool(name="wlq", bufs=2) as wlq,
                tc.tile_pool(name="sscr", bufs=2) as sscr,
                tc.tile_pool(name="xnp", bufs=1) as xnp,
                tc.tile_pool(name="mpsum", bufs=4, space="PSUM") as mpsum,
                tc.tile_pool(name="spool", bufs=4) as spool,
                tc.tile_pool(name="gpool", bufs=8) as gpool,
                tc.tile_pool(name="qscr", bufs=10) as qscr,
            ):

                def w_tile_load(wq, ti):
                    t = wq * (KQ // 128) + ti
                    wt = wload.tile([128, D], F32, tag="wt")
                    nc.sync.dma_start(out=wt, in_=wv[t])
                    return wt

                def w_tile_proc(wq, ti, wt, wl16Tq):
                    t = wq * (KQ // 128) + ti
                    w16n = wstage.tile([128, D], F16, tag="w16n")
                    nc.gpsimd.tensor_copy(out=w16n, in_=wt)
                    sqw = wstage.tile([128, D], F16, tag="sqw")
                    nc.scalar.activation(
                        out=sqw, in_=wt, func=ACTF.Square,
                        accum_out=s_nat[:, t : t + 1],
                    )
                    wl16n = wstage.tile([128, D], F16, tag="wl16n")
                    nc.gpsimd.tensor_sub(out=wl16n, in0=wt, in1=w16n)
                    ksl = slice(t * 128, (t + 1) * 128)
                    ksq = slice(ti * 128, (ti + 1) * 128)
                    nc.sync.dma_start_transpose(wh16[:, :, ksl], w16n)
                    nc.sync.dma_start_transpose(wl16Tq[:, :, ksq], wl16n)

                def w_bulk(wq, wl16Tq):
                    qsl = slice(wq * KQ, (wq + 1) * KQ)
                    nc.gpsimd.tensor_copy(out=wh8[:, :, qsl], in_=wh16[:, :, qsl])
                    nc.gpsimd.tensor_tensor(
                        out=wl8[:, :, qsl], in0=wl16Tq, in1=c256, op=OP.mult
                    )

                NWT = KQ // 128
                def w_prep(wq):
                    wl16Tq = wlq.tile([128, DH, KQ], F16, tag="wl16Tq")
                    wts = [w_tile_load(wq, ti) for ti in range(NWT)]
                    for ti in range(NWT):
                        w_tile_proc(wq, ti, wts[ti], wl16Tq)
                    w_bulk(wq, wl16Tq)

                def w_sdig(wq):
                    # s digits segment -> DRAM -> sdig columns
                    qsl = slice(wq * KQ, (wq + 1) * KQ)
                    tsl = slice(wq * (KQ // 128), (wq + 1) * (KQ // 128))
                    NS16 = KQ // 128
                    t1 = sscr.tile([128, NS16], F32, tag="t1")
                    nc.vector.tensor_scalar(
                        t1, s_nat[:, tsl], -(S / CC), scalar2=None, op0=OP.mult
                    )
                    d1 = sscr.tile([128, NS16], F32, tag="d1")
                    d116 = sscr.tile([128, NS16], F16, tag="d116")
                    nc.vector.tensor_copy(out=d116, in_=t1)
                    nc.vector.tensor_copy(out=d1, in_=d116)
                    d2 = sscr.tile([128, NS16], F32, tag="d2")
                    nc.vector.tensor_sub(out=d2, in0=t1, in1=d1)
                    for i, dsrc in enumerate((d1, d2)):
                        ps_ = mpsum.tile([128, KQ], F32, tag="pq")
                        nc.tensor.transpose(ps_[0:NS16, 0:128], dsrc[:, :], ident[:, :])
                        dT = sscr.tile([NS16, 128], F16, tag="dT")
                        nc.vector.tensor_copy(out=dT, in_=ps_[0:NS16, 0:128])
                        nc.sync.dma_start(
                            out=sd_dram[i, qsl].rearrange("(t p) -> t p", p=128),
                            in_=dT[:, :],
                        )
                    nc.sync.dma_start(out=sdig[:, qsl], in_=sd_dram[:, qsl])

                # quarter 0 w-prep before the BN wait
                w_prep(0)
                w_sdig(0)

                # ---------- bn math (waits on the collective) ----------
                nc.sync.dma_start(
                    out=gb[:, 0:2], in_=gamma[:].rearrange("(h p) -> p h", p=128)
                )
                nc.sync.dma_start(
                    out=gb[:, 2:4], in_=beta[:].rearrange("(h p) -> p h", p=128)
                )
                mean = bn[:, 0:2]
                var = bn[:, 2:4]
                rstd = bn[:, 4:6]
                scale2 = bn[:, 6:8]
                inv_n = 1.0 / float(N)
                nc.vector.tensor_scalar(mean, tots[:, 0:2], inv_n, scalar2=None, op0=OP.mult)
                nc.vector.tensor_scalar(var, tots[:, 2:4], inv_n, scalar2=None, op0=OP.mult)
                msq = tots[:, 0:2]
                nc.vector.tensor_tensor(out=msq, in0=mean, in1=mean, op=OP.mult)
                nc.vector.tensor_tensor(out=var, in0=var, in1=msq, op=OP.subtract)
                nc.vector.tensor_scalar(var, var, BN_EPS, scalar2=None, op0=OP.add)
                nc.vector.reciprocal(out=var, in_=var)
                nc.scalar.activation(out=rstd, in_=var, func=ACTF.Sqrt)
                # scale2 = 2*S*rstd*gamma ; bias2 = 2*S*beta - mean*scale2
                nc.vector.tensor_tensor(out=scale2, in0=rstd, in1=gb[:, 0:2], op=OP.mult)
                nc.vector.tensor_scalar(scale2, scale2, 2.0 * S, scalar2=None, op0=OP.mult)
                bias2 = gb[:, 2:4]
                nc.vector.tensor_scalar(bias2, bias2, 2.0 * S, scalar2=None, op0=OP.mult)
                mscale = tots[:, 0:2]
                nc.vector.tensor_tensor(out=mscale, in0=mean, in1=scale2, op=OP.mult)
                nc.vector.tensor_tensor(out=bias2, in0=bias2, in1=mscale, op=OP.subtract)

                # x-side casts (DVE + ACT)
                for h in range(DH):
                    # fp16 affine directly (rounds on write) + f32 affine for the residual
                    nc.vector.tensor_scalar(
                        xh16[:, h, :], xT[h][:, :],
                        scale2[:, h : h + 1], scalar2=bias2[:, h : h + 1],
                        op0=OP.mult, op1=OP.add,
                    )
                    xn2 = xnp.tile([128, NS], F32, tag="xn2")
                    nc.vector.tensor_scalar(
                        xn2, xT[h][:, :],
                        scale2[:, h : h + 1], scalar2=bias2[:, h : h + 1],
                        op0=OP.mult, op1=OP.add,
                    )
                    nc.vector.tensor_sub(out=xl8[:, h, :], in0=xn2, in1=xh16[:, h, :])
                nc.scalar.activation(out=xh8, in_=xh16, func=ACTF.Copy, scale=AX8)

                # ---------- main loop: quarter-outer ----------
                def combine(nt):
                    csl = slice(nt * NQ, nt * NQ + NQ)
                    qmax = qmaxall[:, csl]
                    qidx = qscr.tile([128, NQ], F32, tag="qidx")
                    m = qscr.tile([128, 1], F32, tag="m")
                    nc.vector.tensor_reduce(m, qmax, axis=AX.X, op=OP.max)
                    # increasing in k*: qidx = qcand - (sum(k) - k*)
                    nc.gpsimd.tensor_tensor(
                        out=qidx, in0=qcand_base, in1=qidxall[:, csl], op=OP.subtract
                    )
                    pen = qscr.tile([128, NQ], F32, tag="pen")
                    nc.vector.tensor_scalar(
                        pen, qmax, m[:, 0:1], scalar2=1e9, op0=OP.is_lt, op1=OP.mult
                    )
                    nc.gpsimd.tensor_tensor(out=qidx, in0=qidx, in1=pen, op=OP.add)
                    idxf = qscr.tile([128, 1], F32, tag="idxf")
                    nc.vector.tensor_reduce(idxf, qidx[:, :], axis=AX.X, op=OP.min)
                    idxi = qscr.tile([128, 1], I32, tag="idxi")
                    nc.vector.tensor_copy(out=idxi, in_=idxf)
                    gath = gpool.tile([128, D], F32, tag="gath")
                    nc.gpsimd.indirect_dma_start(
                        out=gath,
                        out_offset=None,
                        in_=w[:, :],
                        in_offset=bass.IndirectOffsetOnAxis(ap=idxi[:, 0:1], axis=0),
                    )
                    nc.sync.dma_start(out=yv[:, nt, :], in_=gath)

                for q in range(NQ):
                    wl16Tq = None
                    wpend = []
                    if q + 1 < NQ:
                        wl16Tq = wlq.tile([128, DH, KQ], F16, tag="wl16Tq")
                        wpend = [w_tile_load(q + 1, 0), w_tile_load(q + 1, 1)]

                    NWT_ = KQ // 128
                    for nt in range(NT):
                        if wl16Tq is not None:
                            if nt + 2 < NWT_:
                                wpend.append(w_tile_load(q + 1, nt + 2))
                            if nt < NWT_:
                                w_tile_proc(q + 1, nt, wpend[nt], wl16Tq)
                        nsl = slice(nt * 128, (nt + 1) * 128)
                        col = nt * NQ + q
                        pq = mpsum.tile([128, KQ], F32, tag="pq")
                        i = 0
                        # fp16 main term, term-major (stationary reuse)
                        for h in range(DH):
                            for c in range(NCH):
                                kofs = q * KQ + c * 512
                                nc.tensor.matmul(
                                    pq[:, c * 512 : (c + 1) * 512],
                                    xh16[:, h, nsl],
                                    wh16[:, h, kofs : kofs + 512],
                                    start=(i == 0), stop=False,
                                )
                            i += 1
                        # fp8 DoubleRow crosses (contract 256 per instr)
                        for xa, wa in ((xh8, wl8), (xl8, wh8)):
                            for c in range(NCH):
                                kofs = q * KQ + c * 512
                                nc.tensor.matmul(
                                    pq[:, c * 512 : (c + 1) * 512],
                                    xa[:, :, nsl],
                                    wa[:, :, kofs : kofs + 512],
                                    start=False, stop=False,
                                    perf_mode=PM.DoubleRow,
                                )
                        # -S*||w||^2 rank-2 fp16 matmul
                        for c in range(NCH):
                            kofs = q * KQ + c * 512
                            nc.tensor.matmul(
                                pq[:, c * 512 : (c + 1) * 512],
                                cones[:, :],
                                sdig[:, kofs : kofs + 512],
                                start=False, stop=(c == NCH - 1),
                            )

                        # qmax = max(pq) (DVE, from PSUM)
                        nc.vector.tensor_reduce(
                            qmaxall[:, col : col + 1], pq, axis=AX.X, op=OP.max
                        )
                        # index: qidx = sum(k) - k*
                        if (3 * (nt + q)) % 8 < 3:
                            nc.vector.scalar_tensor_tensor(
                                out=junk_d, in0=pq,
                                scalar=qmaxall[:, col : col + 1],
                                in1=iota16,
                                op0=OP.is_lt, op1=OP.mult,
                                accum_out=qidxall[:, col : col + 1],
                            )
                        else:
                            sgn = spool.tile([128, KQ], F16, tag="sgn")
                            nc.scalar.activation(
                                out=sgn, in_=pq, func=ACTF.Sign,
                                bias=qmaxall[:, col : col + 1], scale=-1.0,
                            )
                            t16 = spool.tile([128, KQ], F16, tag="t16")
                            nc.gpsimd.tensor_tensor(
                                out=t16, in0=sgn, in1=iota16, op=OP.mult
                            )
                            nc.scalar.activation(
                                out=junk_a, in_=t16, func=ACTF.Copy,
                                accum_out=qidxall[:, col : col + 1],
                            )
                        if q == NQ - 1:
                            combine(nt)

                    if q + 1 < NQ:
                        w_bulk(q + 1, wl16Tq)
                        w_sdig(q + 1)


    return nc


def _get_nc():
    if "nc" not in _cache:
        nc_ = _build()
        if not nc_.is_finalized():
            nc_.finalize()
        _cache["nc"] = nc_
    return _cache["nc"]


def kernel(x, weight, gamma, beta):
    x = np.ascontiguousarray(x, dtype=np.float32)
    weight = np.ascontiguousarray(weight, dtype=np.float32)
    gamma = np.ascontiguousarray(gamma, dtype=np.float32)
    beta = np.ascontiguousarray(beta, dtype=np.float32)

    nc = _get_nc()
    in_maps = [
        {
            "x": x[c * NS : (c + 1) * NS],
            "w": weight,
            "gamma": gamma,
            "beta": beta,
        }
        for c in range(NCORES)
    ]
    res = run_bass_kernel_spmd(nc, in_maps, list(range(NCORES)))
    return np.concatenate([res.results[c]["y"] for c in range(NCORES)], axis=0)


if __name__ == "__main__":
    _build()
    print("kernel build OK")


# revision 9
# speedup vs baseline: 1.0737x; 1.0674x over previous
"""Trainium2 Bass kernel for nn_NearestEmbedding (vq_codebook) — v3.

reference:
  xn  = BatchNorm1d(x)   (training mode, biased batch stats)
  out = weight[argmin_k ||xn - weight_k||^2]

argmin via maximization of v = 2*xn.w_k - ||w_k||^2 (row-constant dropped).

v is computed at global scale S=2^12 in PSUM:
  v' = xh@wh (fp16 main) + xh8@wl8 + xl8@wh8 (fp8e4m3 DoubleRow crosses,
       0.5 cyc/row => ~2x PE speedup over a 3-term fp16 split)
     + cones@sdig (rank-2 fp16 matmul adding -S*||w_k||^2)
where xh = fp16(2*S*xn), xl = 2*S*xn - xh, wh = fp16(w), wl = w - wh,
xh8 = fp8(xh*2^-8), wl8 = fp8(wl*2^8), xl8 = fp8(xl), wh8 = fp8(wh).
Numpy-verified on the dataset: 0/16384 argmax flips vs the fp32 reference.

Reduction per 2048-wide quarter (all index forms produce sum(k) - k*):
  DVE tensor_reduce(max) from PSUM -> qmax
  q0/q2: DVE scalar_tensor_tensor: sum(1[pq < qmax]*k)
  q1/q3: ACT Sign(qmax - pq) -> {1 below max, 0 at max} (fp16),
         sign*iota on Pool (fp16 mult), ACT Copy+accum_out sums it
Quarter-OUTER loop: w-side prep for quarter q (DMA loads, fp16 cast +
residual, DMA-xbar transposes, fp8 casts, s-digit segment) is emitted
just before the 16 n-tiles of quarter-q matmuls, so w-prep of q+1
overlaps compute of q. Final per-n-tile combine (first-index tie-break,
matching jnp.argmin) selects k*; codebook rows are gathered from DRAM
via indirect DMA (bit-exact output rows).
"""
import sys
sys.path.insert(0, "/opt/trn_rl_repo")
import numpy as np
import concourse.bass as bass
from concourse import bacc
import concourse.mybir as mybir
from concourse.tile import TileContext
from concourse.bass_utils import run_bass_kernel_spmd

F32 = mybir.dt.float32
F16 = mybir.dt.float16
F8 = mybir.dt.float8e4
I32 = mybir.dt.int32
U16 = mybir.dt.uint16
AX = mybir.AxisListType
OP = mybir.AluOpType
ACTF = mybir.ActivationFunctionType
PM = mybir.MatmulPerfMode

NCORES = 8
N, K, D = 16384, 8192, 256
NS = N // NCORES            # 2048 rows per core
NT = NS // 128              # 16 n-tiles
DH = D // 128               # 2 contract halves
KQ = 1024                   # k-quarter (2 psum banks)
NQ = K // KQ                # 8 quarters
NCH = KQ // 512             # 2 matmul chunks per quarter
BN_EPS = 1e-5

S = 4096.0                  # global v scale 2^12
AX8 = 2.0 ** -8             # xh8 = fp8(xh * AX8)
AW8 = 2.0 ** 8              # wl8 = fp8(wl * AW8)
CC = 16384.0                # s-digit matmul carrier

_cache = {}


def _build() -> bass.Bass:
    from concourse.masks import make_identity

    nc = bacc.Bacc("TRN2", target_bir_lowering=False, debug=False, num_devices=NCORES)
    x = nc.dram_tensor("x", [NS, D], F32, kind="ExternalInput")
    w = nc.dram_tensor("w", [K, D], F32, kind="ExternalInput")
    gamma = nc.dram_tensor("gamma", [D], F32, kind="ExternalInput")
    beta = nc.dram_tensor("beta", [D], F32, kind="ExternalInput")
    y = nc.dram_tensor("y", [NS, D], F32, kind="ExternalOutput")

    cc_in = nc.dram_tensor("cc_in", [128, 4], F32)
    cc_out = nc.dram_tensor("cc_out", [NCORES, 128, 4], F32, addr_space="Shared")
    sd_dram = nc.dram_tensor("sd_dram", [2, K], F16)

    wv = w[:, :].rearrange("(t p) d -> t p d", p=128)       # [64, 128, 256]
    xv = x[:, :].rearrange("(t p) d -> t p d", p=128)       # [16, 128, 256]
    yv = y[:, :].rearrange("(t p) d -> p t d", p=128)       # [128, 16, 256]

    with TileContext(nc) as tc:
        with (
            tc.tile_pool(name="const", bufs=1) as constp,
            tc.tile_pool(name="big", bufs=1) as big,
            tc.tile_pool(name="small", bufs=1) as small,
        ):
            ident = constp.tile([128, 128], F32, tag="ident")
            make_identity(nc, ident[:, :])

            # iota16[p, k] = k for k in 0..KQ-1 (exact in fp16 up to 2048)
            iota16 = constp.tile([128, KQ], F16, tag="iota16")
            with tc.tile_pool(name="iotascr", bufs=1) as iotascr:
                iotau = iotascr.tile([128, KQ], U16, tag="iotau")
                nc.gpsimd.iota(iotau, pattern=[[1, KQ]], base=0, channel_multiplier=0)
                nc.vector.tensor_copy(out=iota16, in_=iotau)

            # persistent tiles
            wh16 = big.tile([128, DH, K], F16, tag="wh16")    # fp16(w) transposed
            wh8 = big.tile([128, DH, K], F8, tag="wh8")       # fp8(wh)
            wl8 = big.tile([128, DH, K], F8, tag="wl8")       # fp8(wl*2^8)
            sdig = big.tile([2, K], F16, tag="sdig")          # -S*s/CC digits
            cones = constp.tile([2, 128], F16, tag="cones")
            nc.vector.memset(cones[:, :], CC)
            c256 = constp.tile([128, DH, KQ], F16, tag="c256")
            nc.vector.memset(c256[:, :, :], AW8)
            xh16 = big.tile([128, DH, NS], F16, tag="xh16")   # fp16(2*S*xn) transp
            xh8 = big.tile([128, DH, NS], F8, tag="xh8")      # fp8(xh*2^-8)
            xl8 = big.tile([128, DH, NS], F8, tag="xl8")      # fp8(2*S*xn - xh)
            xT = [big.tile([128, NS], F32, tag=f"xT{h}", name=f"xT{h}") for h in range(DH)]
            junk_d = big.tile([128, KQ], F16, tag="junk_d")
            junk_a = big.tile([128, KQ], F16, tag="junk_a")

            s_nat = small.tile([128, K // 128], F32, tag="s_nat")   # [128, 64]
            stats = small.tile([128, 4], F32, tag="stats")
            tots = small.tile([128, 4], F32, tag="tots")
            gb = small.tile([128, 4], F32, tag="gb")
            bn = small.tile([128, 8], F32, tag="bn")
            qmaxall = small.tile([128, NT * NQ], F32, tag="qmaxall")
            qidxall = small.tile([128, NT * NQ], F32, tag="qidxall")
            # all index forms produce sum(k)-k*; base = q*KQ + sum(k)
            SUMK = float(KQ * (KQ - 1) // 2)
            qcand_base = small.tile([128, NQ], F32, tag="qcb")
            for q in range(NQ):
                nc.vector.memset(qcand_base[:, q : q + 1], float(q * KQ) + SUMK)

            # ---------- x-phase: load, transpose (PE), BN stats ----------
            with (
                tc.tile_pool(name="xload", bufs=6) as xload,
                tc.tile_pool(name="tps", bufs=4, space="PSUM") as tps,
                tc.tile_pool(name="scr2", bufs=2) as scr2,
            ):
                # batch 4 tiles per psum bank per half, one bulk evict each
                for g in range(NT // 4):
                    xts = []
                    for i in range(4):
                        xt = xload.tile([128, D], F32, tag="xt")
                        nc.sync.dma_start(out=xt, in_=xv[g * 4 + i])
                        xts.append(xt)
                    for h in range(DH):
                        ps4 = tps.tile([128, 512], F32, tag="ps4")
                        for i in range(4):
                            nc.tensor.transpose(
                                ps4[:, i * 128 : (i + 1) * 128],
                                xts[i][:, h * 128 : (h + 1) * 128],
                                ident[:, :],
                            )
                        nc.vector.tensor_copy(
                            out=xT[h][:, g * 512 : (g + 1) * 512], in_=ps4
                        )

                for h in range(DH):
                    nc.vector.tensor_reduce(
                        stats[:, h : h + 1], xT[h][:, :], axis=AX.X, op=OP.add
                    )
                    sq2 = scr2.tile([128, NS], F32, tag="sq2")
                    nc.scalar.activation(
                        out=sq2, in_=xT[h][:, :], func=ACTF.Square,
                        accum_out=stats[:, 2 + h : 3 + h],
                    )

                # launch BN stats AllGather (cheaper than AllReduce);
                # the 8 gathered copies are summed locally below
                nc.sync.dma_start(out=cc_in[:, :], in_=stats)
                nc.gpsimd.collective_compute(
                    "AllGather", OP.bypass,
                    replica_groups=[list(range(NCORES))],
                    ins=[cc_in[:, :]], outs=[cc_out[:, :, :]],
                )
                gat = small.tile([128, NCORES, 4], F32, tag="gat")
                nc.sync.dma_start(
                    out=gat, in_=cc_out[:, :, :].rearrange("r p j -> p r j")
                )
                nc.vector.tensor_reduce(
                    tots, gat[:, :, :].rearrange("p r j -> p j r"),
                    axis=AX.X, op=OP.add,
                )

            with (
                tc.tile_pool(name="wload", bufs=8) as wload,
                tc.tile_pool(name="wstage", bufs=10) as wstage,
                tc.tile_pool(name="wlq", bufs=2) as wlq,
                tc.tile_pool(name="sscr", bufs=2) as sscr,
                tc.tile_pool(name="xnp", bufs=1) as xnp,
                tc.tile_pool(name="mpsum", bufs=4, space="PSUM") as mpsum,
                tc.tile_pool(name="spool", bufs=4) as spool,
                tc.tile_pool(name="gpool", bufs=8) as gpool,
                tc.tile_pool(name="qscr", bufs=10) as qscr,
            ):

                def w_tile_load(wq, ti):
                    t = wq * (KQ // 128) + ti
                    wt = wload.tile([128, D], F32, tag="wt")
                    nc.sync.dma_start(out=wt, in_=wv[t])
                    return wt

                def w_tile_proc(wq, ti, wt, wl16Tq):
                    t = wq * (KQ // 128) + ti
                    w16n = wstage.tile([128, D], F16, tag="w16n")
                    nc.gpsimd.tensor_copy(out=w16n, in_=wt)
                    sqw = wstage.tile([128, D], F16, tag="sqw")
                    nc.scalar.activation(
                        out=sqw, in_=wt, func=ACTF.Square,
                        accum_out=s_nat[:, t : t + 1],
                    )
                    wl16n = wstage.tile([128, D], F16, tag="wl16n")
                    nc.gpsimd.tensor_sub(out=wl16n, in0=wt, in1=w16n)
                    ksl = slice(t * 128, (t + 1) * 128)
                    ksq = slice(ti * 128, (ti + 1) * 128)
                    nc.sync.dma_start_transpose(wh16[:, :, ksl], w16n)
                    nc.sync.dma_start_transpose(wl16Tq[:, :, ksq], wl16n)

                def w_bulk(wq, wl16Tq):
                    qsl = slice(wq * KQ, (wq + 1) * KQ)
                    nc.gpsimd.tensor_copy(out=wh8[:, :, qsl], in_=wh16[:, :, qsl])
                    nc.gpsimd.tensor_tensor(
                        out=wl8[:, :, qsl], in0=wl16Tq, in1=c256, op=OP.mult
                    )

                NWT = KQ // 128
                def w_prep(wq):
                    wl16Tq = wlq.tile([128, DH, KQ], F16, tag="wl16Tq")
                    wts = [w_tile_load(wq, ti) for ti in range(NWT)]
                    for ti in range(NWT):
                        w_tile_proc(wq, ti, wts[ti], wl16Tq)
                    w_bulk(wq, wl16Tq)

                def w_sdig(wq):
                    # s digits segment -> DRAM -> sdig columns
                    qsl = slice(wq * KQ, (wq + 1) * KQ)
                    tsl = slice(wq * (KQ // 128), (wq + 1) * (KQ // 128))
                    NS16 = KQ // 128
                    t1 = sscr.tile([128, NS16], F32, tag="t1")
                    nc.vector.tensor_scalar(
                        t1, s_nat[:, tsl], -(S / CC), scalar2=None, op0=OP.mult
                    )
                    d1 = sscr.tile([128, NS16], F32, tag="d1")
                    d116 = sscr.tile([128, NS16], F16, tag="d116")
                    nc.vector.tensor_copy(out=d116, in_=t1)
                    nc.vector.tensor_copy(out=d1, in_=d116)
                    d2 = sscr.tile([128, NS16], F32, tag="d2")
                    nc.vector.tensor_sub(out=d2, in0=t1, in1=d1)
                    for i, dsrc in enumerate((d1, d2)):
                        ps_ = mpsum.tile([128, KQ], F32, tag="pq")
                        nc.tensor.transpose(ps_[0:NS16, 0:128], dsrc[:, :], ident[:, :])
                        dT = sscr.tile([NS16, 128], F16, tag="dT")
                        nc.vector.tensor_copy(out=dT, in_=ps_[0:NS16, 0:128])
                        nc.sync.dma_start(
                            out=sd_dram[i, qsl].rearrange("(t p) -> t p", p=128),
                            in_=dT[:, :],
                        )
                    nc.sync.dma_start(out=sdig[:, qsl], in_=sd_dram[:, qsl])

                # quarter 0 w-prep before the BN wait
                w_prep(0)
                w_sdig(0)

                # ---------- bn math (waits on the collective) ----------
                nc.sync.dma_start(
                    out=gb[:, 0:2], in_=gamma[:].rearrange("(h p) -> p h", p=128)
                )
                nc.sync.dma_start(
                    out=gb[:, 2:4], in_=beta[:].rearrange("(h p) -> p h", p=128)
                )
                mean = bn[:, 0:2]
                var = bn[:, 2:4]
                rstd = bn[:, 4:6]
                scale2 = bn[:, 6:8]
                inv_n = 1.0 / float(N)
                nc.vector.tensor_scalar(mean, tots[:, 0:2], inv_n, scalar2=None, op0=OP.mult)
                nc.vector.tensor_scalar(var, tots[:, 2:4], inv_n, scalar2=None, op0=OP.mult)
                msq = tots[:, 0:2]
                nc.vector.tensor_tensor(out=msq, in0=mean, in1=mean, op=OP.mult)
                nc.vector.tensor_tensor(out=var, in0=var, in1=msq, op=OP.subtract)
                nc.vector.tensor_scalar(var, var, BN_EPS, scalar2=None, op0=OP.add)
                nc.vector.reciprocal(out=var, in_=var)
                nc.scalar.activation(out=rstd, in_=var, func=ACTF.Sqrt)
                # scale2 = 2*S*rstd*gamma ; bias2 = 2*S*beta - mean*scale2
                nc.vector.tensor_tensor(out=scale2, in0=rstd, in1=gb[:, 0:2], op=OP.mult)
                nc.vector.tensor_scalar(scale2, scale2, 2.0 * S, scalar2=None, op0=OP.mult)
                bias2 = gb[:, 2:4]
                nc.vector.tensor_scalar(bias2, bias2, 2.0 * S, scalar2=None, op0=OP.mult)
                mscale = tots[:, 0:2]
                nc.vector.tensor_tensor(out=mscale, in0=mean, in1=scale2, op=OP.mult)
                nc.vector.tensor_tensor(out=bias2, in0=bias2, in1=mscale, op=OP.subtract)

                # x-side casts (DVE + ACT)
                for h in range(DH):
                    # fp16 affine directly (rounds on write) + f32 affine for the residual
                    nc.vector.tensor_scalar(
                        xh16[:, h, :], xT[h][:, :],
                        scale2[:, h : h + 1], scalar2=bias2[:, h : h + 1],
                        op0=OP.mult, op1=OP.add,
                    )
                    xn2 = xnp.tile([128, NS], F32, tag="xn2")
                    nc.vector.tensor_scalar(
                        xn2, xT[h][:, :],
                        scale2[:, h : h + 1], scalar2=bias2[:, h : h + 1],
                        op0=OP.mult, op1=OP.add,
                    )
                    nc.vector.tensor_sub(out=xl8[:, h, :], in0=xn2, in1=xh16[:, h, :])
                nc.scalar.activation(out=xh8, in_=xh16, func=ACTF.Copy, scale=AX8)

                # ---------- main loop: quarter-outer ----------
                def combine(nt):
                    csl = slice(nt * NQ, nt * NQ + NQ)
                    qmax = qmaxall[:, csl]
                    qidx = qscr.tile([128, NQ], F32, tag="qidx")
                    m = qscr.tile([128, 1], F32, tag="m")
                    nc.vector.tensor_reduce(m, qmax, axis=AX.X, op=OP.max)
                    # increasing in k*: qidx = qcand - (sum(k) - k*)
                    nc.gpsimd.tensor_tensor(
                        out=qidx, in0=qcand_base, in1=qidxall[:, csl], op=OP.subtract
                    )
                    pen = qscr.tile([128, NQ], F32, tag="pen")
                    nc.vector.tensor_scalar(
                        pen, qmax, m[:, 0:1], scalar2=1e9, op0=OP.is_lt, op1=OP.mult
                    )
                    nc.gpsimd.tensor_tensor(out=qidx, in0=qidx, in1=pen, op=OP.add)
                    idxf = qscr.tile([128, 1], F32, tag="idxf")
                    nc.vector.tensor_reduce(idxf, qidx[:, :], axis=AX.X, op=OP.min)
                    idxi = qscr.tile([128, 1], I32, tag="idxi")
                    nc.vector.tensor_copy(out=idxi, in_=idxf)
                    gath = gpool.tile([128, D], F32, tag="gath")
                    nc.gpsimd.indirect_dma_start(
                        out=gath,
                        out_offset=None,
                        in_=w[:, :],
                        in_offset=bass.IndirectOffsetOnAxis(ap=idxi[:, 0:1], axis=0),
                    )
                    nc.sync.dma_start(out=yv[:, nt, :], in_=gath)

                for q in range(NQ):
                    wl16Tq = None
                    wpend = []
                    if q + 1 < NQ:
                        wl16Tq = wlq.tile([128, DH, KQ], F16, tag="wl16Tq")
                        wpend = [w_tile_load(q + 1, 0), w_tile_load(q + 1, 1)]

                    NWT_ = KQ // 128
                    for nt in range(NT):
                        if wl16Tq is not None:
                            if nt + 2 < NWT_:
                                wpend.append(w_tile_load(q + 1, nt + 2))
                            if nt < NWT_:
                                w_tile_proc(q + 1, nt, wpend[nt], wl16Tq)
                        nsl = slice(nt * 128, (nt + 1) * 128)
                        col = nt * NQ + q
                        pq = mpsum.tile([128, KQ], F32, tag="pq")
                        i = 0
                        # fp16 main term, term-major (stationary reuse)
                        for h in range(DH):
                            for c in range(NCH):
                                kofs = q * KQ + c * 512
                                nc.tensor.matmul(
                                    pq[:, c * 512 : (c + 1) * 512],
                                    xh16[:, h, nsl],
                                    wh16[:, h, kofs : kofs + 512],
                                    start=(i == 0), stop=False,
                                )
                            i += 1
                        # fp8 DoubleRow crosses (contract 256 per instr)
                        for xa, wa in ((xh8, wl8), (xl8, wh8)):
                            for c in range(NCH):
                                kofs = q * KQ + c * 512
                                nc.tensor.matmul(
                                    pq[:, c * 512 : (c + 1) * 512],
                                    xa[:, :, nsl],
                                    wa[:, :, kofs : kofs + 512],
                                    start=False, stop=False,
                                    perf_mode=PM.DoubleRow,
                                )
                        # -S*||w||^2 rank-2 fp16 matmul
                        for c in range(NCH):
                            kofs = q * KQ + c * 512
                            nc.tensor.matmul(
                                pq[:, c * 512 : (c + 1) * 512],
                                cones[:, :],
                                sdig[:, kofs : kofs + 512],
                                start=False, stop=(c == NCH - 1),
                            )

                        # qmax = max(pq) (DVE, from PSUM)
                        nc.vector.tensor_reduce(
                            qmaxall[:, col : col + 1], pq, axis=AX.X, op=OP.max
                        )
                        # index: qidx = sum(k) - k*
                        if (3 * (nt + q)) % 8 < 3:
                            nc.vector.scalar_tensor_tensor(
                                out=junk_d, in0=pq,
                                scalar=qmaxall[:, col : col + 1],
                                in1=iota16,
                                op0=OP.is_lt, op1=OP.mult,
                                accum_out=qidxall[:, col : col + 1],
                            )
                        else:
                            sgn = spool.tile([128, KQ], F16, tag="sgn")
                            nc.scalar.activation(
                                out=sgn, in_=pq, func=ACTF.Sign,
                                bias=qmaxall[:, col : col + 1], scale=-1.0,
                            )
                            t16 = spool.tile([128, KQ], F16, tag="t16")
                            nc.gpsimd.tensor_tensor(
                                out=t16, in0=sgn, in1=iota16, op=OP.mult
                            )
                            nc.scalar.activation(
                                out=junk_a, in_=t16, func=ACTF.Copy,
                                accum_out=qidxall[:, col : col + 1],
                            )
                        if q == NQ - 1:
                            combine(nt)

                    if q + 1 < NQ:
                        w_bulk(q + 1, wl16Tq)
                        w_sdig(q + 1)


    return nc


def _get_nc():
    if "nc" not in _cache:
        nc_ = _build()
        if not nc_.is_finalized():
            nc_.finalize()
        _cache["nc"] = nc_
    return _cache["nc"]


def kernel(x, weight, gamma, beta):
    x = np.ascontiguousarray(x, dtype=np.float32)
    weight = np.ascontiguousarray(weight, dtype=np.float32)
    gamma = np.ascontiguousarray(gamma, dtype=np.float32)
    beta = np.ascontiguousarray(beta, dtype=np.float32)

    nc = _get_nc()
    in_maps = [
        {
            "x": x[c * NS : (c + 1) * NS],
            "w": weight,
            "gamma": gamma,
            "beta": beta,
        }
        for c in range(NCORES)
    ]
    res = run_bass_kernel_spmd(nc, in_maps, list(range(NCORES)))
    return np.concatenate([res.results[c]["y"] for c in range(NCORES)], axis=0)


if __name__ == "__main__":
    _build()
    print("kernel build OK")
